# revision 17
# baseline (speedup 1.0000x reference)
"""LSTM greedy decoder on 8 trn2 NeuronCores.

Sharding: vocab-parallel. Each core keeps a resident SBUF copy of its
4000-row W_out shard, replicates the LSTM cell, and exchanges per-step
(max, argmax) candidates via a tiny AllGather to agree on the greedy token.

Speed/precision split: the vocab projection runs in fp32r (TF32-class,
~4x faster on the PE); the greedy feedback is protected by re-evaluating
the global top-3 candidate logits exactly in fp32 (DVE dot products)
before picking the token. The LSTM recurrence itself stays exact fp32.
The x @ W_ih.T + biases term is folded into a host-precomputed table
(embed_table @ W_ih.T + b) gathered per token and added into PSUM.
"""

import numpy as np

B, H, D, V, S = 64, 512, 256, 32000, 64
NCORES = 8
VS = V // NCORES            # 4000 vocab rows per core
G4 = 4 * H                  # 2048 gate units
NT = 8                      # logits N-tiles per step
TN = VS // NT               # 500 columns per logits tile
KH = H // 128               # 4 contraction tiles over H
BIG = 1.0e9
NCAND = 3                   # exact-rechecked candidates per level
TC = 2                      # candidates shipped per logits tile


def build_program(steps=S):
    import concourse.bass as bass
    import concourse.bacc as bacc
    import concourse.mybir as mybir
    import concourse.tile as tile
    from concourse.masks import make_identity

    f32 = mybir.dt.float32
    f32r = mybir.dt.float32r
    u32 = mybir.dt.uint32
    AF = mybir.ActivationFunctionType
    OP = mybir.AluOpType
    AX = mybir.AxisListType

    nc = bacc.Bacc(num_devices=NCORES)
    h0T_p = nc.declare_dram_parameter("h0T", [H, B], f32, isOutput=False)
    c0_p = nc.declare_dram_parameter("c0", [B, H], f32, isOutput=False)
    whhT_p = nc.declare_dram_parameter("whhT", [H, G4], f32, isOutput=False)
    bias_p = nc.declare_dram_parameter("bias", [1, G4], f32, isOutput=False)
    woutT_p = nc.declare_dram_parameter("woutT", [H, VS], f32, isOutput=False)
    bout_p = nc.declare_dram_parameter("bout", [1, VS], f32, isOutput=False)
    whe_p = nc.declare_dram_parameter("whe", [V, G4], f32, isOutput=False)
    wfullb_p = nc.declare_dram_parameter("wfullb", [V, H + 1], f32, isOutput=False)
    basec_p = nc.declare_dram_parameter("basec", [B, 1], f32, isOutput=False)
    out_p = nc.declare_dram_parameter("out", [steps, B, VS], f32, isOutput=True)

    rg = [list(range(NCORES))]

    with tile.TileContext(nc) as tc:
        with (
            tc.tile_pool(name="wpool", bufs=1) as wp,
            tc.tile_pool(name="state", bufs=2) as sp,
            tc.tile_pool(name="work", bufs=2) as kp,
            tc.tile_pool(name="ps_g", bufs=1, space="PSUM") as pg,
            tc.tile_pool(name="ps_l", bufs=2, space="PSUM") as pl,
            tc.tile_pool(name="ps_t", bufs=2, space="PSUM") as pt,
            tc.tile_pool(name="dram", bufs=2, space="DRAM") as dp,
        ):
            # ---- constants (engine-local, no DMA) ----
            ident = wp.tile([128, 128], f32)
            make_identity(nc, ident[:])
            ones1 = wp.tile([1, B], f32)
            nc.vector.memset(ones1[:], 1.0)
            ones1r = wp.tile([1, B], f32r)
            nc.vector.tensor_copy(ones1r[:], ones1[:])

            # ---- resident weights (barriers cap per-inst sync-wait fan-in) ----
            tc.strict_bb_all_engine_barrier()
            whh = wp.tile([128, KH, G4], f32)
            nc.sync.dma_start(out=whh[:], in_=whhT_p[:].rearrange("(a p) n -> p a n", p=128))
            # one-shot t=0 bias lives in the same slot the per-step gx reuses
            bias = kp.tile([1, G4], f32, tag="gx", bufs=1, name="bias")
            nc.sync.dma_start(out=bias[:], in_=bias_p[:])
            tc.strict_bb_all_engine_barrier()

            # fp32r weights: stage fp32 chunks through the logits-tagged slot,
            # then round-copy (walrus requires fp32r-matmul inputs pre-rounded)
            wout = wp.tile([128, KH, VS], f32r)
            bout = wp.tile([1, VS], f32r)
            wq = woutT_p[:].rearrange("(a p) n -> p a n", p=128)
            for k in range(KH):
                stage = kp.tile([128, VS], f32, tag="logits", name=f"wstage{k}")
                nc.sync.dma_start(out=stage[:], in_=wq[:, k, :])
                nc.vector.tensor_copy(wout[:, k, :], stage[:])
                if k == 0:
                    bstage = kp.tile([1, VS], f32, tag="logits", name="bstage")
                    nc.sync.dma_start(out=bstage[:], in_=bout_p[:])
                    nc.vector.tensor_copy(bout[:], bstage[:])
                tc.strict_bb_all_engine_barrier()

            basec = wp.tile([B, 1], f32)
            nc.sync.dma_start(out=basec[:], in_=basec_p[:])
            tbg = wp.tile([B, NT * TC], f32)  # global base of tile n = n*TN + core_base
            for n in range(NT):
                nc.vector.memset(tbg[:, TC * n : TC * (n + 1)], float(n * TN))
            nc.vector.tensor_scalar(tbg[:], tbg[:], basec[:, 0:1], None, op0=OP.add)
            hT = sp.tile([128, KH, B], f32, tag="hT")
            nc.sync.dma_start(out=hT[:], in_=h0T_p[:].rearrange("(a p) b -> p a b", p=128))
            tc.strict_bb_all_engine_barrier()
            hTr = sp.tile([128, KH, B], f32r, tag="hTr")
            nc.vector.tensor_copy(hTr[:], hT[:])
            c_prev = sp.tile([B, H], f32, tag="c")
            nc.sync.dma_start(out=c_prev[:], in_=c0_p[:])
            tc.strict_bb_all_engine_barrier()

            def select_topk(vals, idxs, m8, tag):
                """Top-NCAND (value desc, index asc) with distinct indices.
                vals/idxs: [B, ...] f32 APs; m8: [B, 8] sorted maxes of vals.
                Returns ([B, NCAND] vals, [B, NCAND] idxs)."""
                shape = list(vals.shape)
                red_ax = {2: AX.X, 3: AX.XY, 4: AX.XYZ}[len(shape)]
                sv = kp.tile([B, NCAND], f32, tag=f"sv_{tag}", bufs=1, name=f"sv{tag}")
                si = kp.tile([B, NCAND], f32, tag=f"si_{tag}", bufs=1, name=f"si{tag}")
                nc.vector.tensor_copy(sv[:], m8[:, 0:NCAND])
                mask = kp.tile(shape, f32, tag=f"mk_{tag}", bufs=1, name=f"mk{tag}")
                ne = kp.tile(shape, f32, tag=f"ne_{tag}", bufs=1, name=f"ne{tag}")
                cand = kp.tile(shape, f32, tag=f"cd_{tag}", bufs=1, name=f"cd{tag}")
                for k in range(NCAND):
                    nc.vector.tensor_scalar(mask[:], vals[:], m8[:, k : k + 1], None, op0=OP.is_ge)
                    for j in range(k):
                        nc.vector.tensor_scalar(ne[:], idxs[:], si[:, j : j + 1], None, op0=OP.not_equal)
                        nc.vector.tensor_tensor(mask[:], mask[:], ne[:], op=OP.mult)
                    nc.vector.tensor_scalar(cand[:], mask[:], -BIG, BIG, op0=OP.mult, op1=OP.add)
                    nc.vector.tensor_tensor(cand[:], idxs[:], cand[:], op=OP.add)
                    nc.vector.tensor_reduce(si[:, k : k + 1], cand[:], axis=red_ax, op=OP.min)
                return sv, si

            gx = None  # gathered (x @ W_ih.T + b) rows, [B, G4]
            for t in range(steps):
                # ---- gates: h-part on PE (exact fp32); x-part DVE-added ----
                gates = pg.tile([B, G4], f32, tag="gates")
                for n in range(4):
                    ns = slice(n * 512, (n + 1) * 512)
                    if gx is None:  # t == 0: x is zero, init with biases
                        nc.tensor.matmul(out=gates[:, ns], lhsT=ones1[:], rhs=bias[:, ns],
                                         start=True, stop=False)
                    for k in range(KH):
                        nc.tensor.matmul(out=gates[:, ns], lhsT=hT[:, k, :], rhs=whh[:, k, ns],
                                         start=(gx is not None and k == 0), stop=(k == KH - 1))
                if gx is not None:
                    for n in range(4):
                        ns = slice(n * 512, (n + 1) * 512)
                        nc.vector.tensor_tensor(gates[:, ns], gates[:, ns], gx[:, ns], op=OP.add)

                # ---- LSTM pointwise (gate order i, f, g, o) ----
                si_t = kp.tile([B, H], f32, tag="si", bufs=1)
                sf = kp.tile([B, H], f32, tag="sf", bufs=1)
                tg = kp.tile([B, H], f32, tag="tg", bufs=1)
                so = kp.tile([B, H], f32, tag="so", bufs=1)
                nc.scalar.activation(si_t[:], gates[:, 0:512], AF.Sigmoid)
                nc.scalar.activation(sf[:], gates[:, 512:1024], AF.Sigmoid)
                nc.scalar.activation(tg[:], gates[:, 1024:1536], AF.Tanh)
                nc.scalar.activation(so[:], gates[:, 1536:2048], AF.Sigmoid)
                fc = kp.tile([B, H], f32, tag="fc", bufs=1)
                nc.vector.tensor_tensor(fc[:], sf[:], c_prev[:], op=OP.mult)
                ig = kp.tile([B, H], f32, tag="ig", bufs=1)
                nc.vector.tensor_tensor(ig[:], si_t[:], tg[:], op=OP.mult)
                c_new = sp.tile([B, H], f32, tag="c")
                nc.vector.tensor_tensor(c_new[:], fc[:], ig[:], op=OP.add)
                tct = kp.tile([B, H], f32, tag="tct", bufs=1)
                nc.scalar.activation(tct[:], c_new[:], AF.Tanh)
                h1a = kp.tile([B, H + 1], f32, tag="h1")
                h1 = h1a[:, 0:H]
                nc.vector.tensor_tensor(h1[:], so[:], tct[:], op=OP.mult)
                nc.vector.memset(h1a[:, H : H + 1], 1.0)
                c_prev = c_new

                # ---- h1 -> h1.T tiles (PE transpose); fp32 + fp32r copies ----
                hT = sp.tile([128, KH, B], f32, tag="hT")
                hTr = sp.tile([128, KH, B], f32r, tag="hTr")
                for k in range(KH):
                    tp = pt.tile([128, B], f32, tag="tp")
                    nc.tensor.transpose(out=tp[:], in_=h1[:, k * 128 : (k + 1) * 128],
                                        identity=ident[0:B, 0:B])
                    nc.vector.tensor_copy(hT[:, k, :], tp[:])
                    nc.vector.tensor_copy(hTr[:, k, :], tp[:])

                # ---- logits shard in fp32r + per-tile top-2 candidates ----
                logits = kp.tile([B, VS], f32, tag="logits")
                tv = kp.tile([B, NT * TC], f32, tag="tv")
                ti = kp.tile([B, NT * TC], f32, tag="ti")
                m8 = kp.tile([B, 8], f32, tag="m8", bufs=1)
                i8 = kp.tile([B, 8], u32, tag="i8", bufs=1)
                for n in range(NT):
                    ns = slice(n * TN, (n + 1) * TN)
                    lg = pl.tile([B, TN], f32, tag="lg")
                    nc.tensor.matmul(out=lg[:], lhsT=ones1r[:], rhs=bout[:, ns],
                                     start=True, stop=False)
                    for k in range(KH):
                        nc.tensor.matmul(out=lg[:], lhsT=hTr[:, k, :], rhs=wout[:, k, ns],
                                         start=False, stop=(k == KH - 1))
                    nc.scalar.activation(logits[:, ns], lg[:], AF.Copy)
                    if t < steps - 1:
                        cs = slice(TC * n, TC * (n + 1))
                        nc.vector.max(out=m8[:], in_=logits[:, ns])
                        nc.vector.tensor_copy(tv[:, cs], m8[:, 0:TC])
                        nc.vector.max_index(out=i8[:], in_max=m8[:], in_values=logits[:, ns])
                        nc.vector.tensor_copy(ti[:, cs], i8[:, 0:TC])

                nc.sync.dma_start(out=out_p[t], in_=logits[:])
                if t == steps - 1:
                    break

                # ---- ship all 16 per-tile candidates; one AllGather ----
                NPC = NT * TC
                nc.vector.tensor_tensor(ti[:], ti[:], tbg[:], op=OP.add)
                agin = dp.tile([2 * NPC, B], f32, tag="agin")
                nc.sync.dma_start(out=agin[0:NPC, :].rearrange("k b -> b k"), in_=tv[:])
                nc.sync.dma_start(out=agin[NPC : 2 * NPC, :].rearrange("k b -> b k"), in_=ti[:])
                agout = dp.tile([NCORES * 2 * NPC, B], f32, tag="agout", addr_space="Shared")
                nc.gpsimd.collective_compute(
                    "AllGather", OP.bypass, replica_groups=rg,
                    ins=[agin[:].opt()], outs=[agout[:].opt()],
                )
                gall = kp.tile([B, NCORES, 2 * NPC], f32, tag="gall")
                ago = agout[:].rearrange("(r s) b -> b r s", s=2 * NPC)
                nc.sync.dma_start(out=gall[:], in_=ago[:])
                gv = gall[:, :, 0:NPC]
                gi = gall[:, :, NPC : 2 * NPC]

                # ---- global top-3 candidates ----
                gm8 = kp.tile([B, 8], f32, tag="gm8", bufs=1)
                nc.vector.max(out=gm8[:], in_=gv)
                _, gci = select_topk(gv, gi, gm8, "g")

                # ---- exact fp32 recheck of the 3 candidates ----
                ev = kp.tile([B, 8], f32, tag="ev", bufs=1)
                nc.vector.memset(ev[:], -BIG)
                cu = kp.tile([B, NCAND], u32, tag="cu", bufs=1)
                nc.vector.tensor_copy(cu[:], gci[:])
                wrow = kp.tile([B, H + 1], f32, tag="wrow", bufs=2)
                prod = kp.tile([B, H + 1], f32, tag="prod", bufs=1)
                for k in range(NCAND):
                    nc.gpsimd.indirect_dma_start(
                        out=wrow[:], out_offset=None, in_=wfullb_p[:],
                        in_offset=bass.IndirectOffsetOnAxis(ap=cu[:, k : k + 1], axis=0),
                    )
                    nc.vector.tensor_tensor(prod[:], h1a[:], wrow[:], op=OP.mult)
                    nc.vector.tensor_reduce(ev[:, k : k + 1], prod[:], axis=AX.X, op=OP.add)

                em8 = kp.tile([B, 8], f32, tag="em8", bufs=1)
                nc.vector.max(out=em8[:], in_=ev[:])
                emask = kp.tile([B, NCAND], f32, tag="emask", bufs=1)
                nc.vector.tensor_scalar(emask[:], ev[:, 0:NCAND], em8[:, 0:1], None, op0=OP.is_ge)
                nc.vector.tensor_scalar(emask[:], emask[:], -BIG, BIG, op0=OP.mult, op1=OP.add)
                nc.vector.tensor_tensor(emask[:], gci[:], emask[:], op=OP.add)
                gidxf = kp.tile([B, 1], f32, tag="gidxf", bufs=1)
                nc.vector.tensor_reduce(gidxf[:], emask[:], axis=AX.X, op=OP.min)
                gidx = kp.tile([B, 1], u32, tag="gidx")
                nc.vector.tensor_copy(gidx[:], gidxf[:])

                # ---- gather the token's precomputed gate row ----
                gx = kp.tile([B, G4], f32, tag="gx", bufs=1)
                nc.gpsimd.indirect_dma_start(
                    out=gx[:], out_offset=None, in_=whe_p[:],
                    in_offset=bass.IndirectOffsetOnAxis(ap=gidx[:, :1], axis=0),
                )

    nc.finalize()  # Bacc: runs compile() legalization passes
    return nc


def make_in_maps(inputs):
    inp = {k: np.asarray(v) for k, v in inputs.items()}
    h0 = inp["h0"].astype(np.float32)
    c0 = inp["c0"].astype(np.float32)
    W_ih = inp["W_ih"].astype(np.float32)
    W_hh = inp["W_hh"].astype(np.float32)
    b = (inp["b_ih"].astype(np.float32) + inp["b_hh"].astype(np.float32)).reshape(1, G4)
    W_out = inp["W_out"].astype(np.float32)
    b_out = inp["b_out"].astype(np.float32)
    emb = inp["embed_table"].astype(np.float32)
    # x @ W_ih.T + b for every vocab row, fp32
    whe = (emb @ W_ih.T + b).astype(np.float32)
    wfullb = np.ascontiguousarray(
        np.concatenate([W_out, b_out.reshape(V, 1)], axis=1).astype(np.float32))
    in_maps = []
    for c in range(NCORES):
        base = c * VS
        in_maps.append({
            "h0T": np.ascontiguousarray(h0.T),
            "c0": np.ascontiguousarray(c0),
            "whhT": np.ascontiguousarray(W_hh.T),
            "bias": b,
            "woutT": np.ascontiguousarray(W_out[base : base + VS].T),
            "bout": np.ascontiguousarray(b_out[base : base + VS].reshape(1, VS)),
            "whe": whe,
            "wfullb": wfullb,
            "basec": np.full((B, 1), float(base), np.float32),
        })
    return in_maps


def run(inputs, steps=S, trace=False):
    from concourse.bass_utils import run_bass_kernel_spmd

    nc = build_program(steps)
    res = run_bass_kernel_spmd(nc, make_in_maps(inputs), list(range(NCORES)),
                               trace=trace)
    outs = [res.results[c]["out"] for c in range(NCORES)]      # each [steps, B, VS]
    full = np.concatenate(outs, axis=2)                        # [steps, B, V]
    return np.ascontiguousarray(np.transpose(full, (1, 0, 2))), res


def kernel(**inputs):
    out, _ = run(inputs, steps=S, trace=False)
    return out.astype(np.float32)
